# revision 19
# baseline (speedup 1.0000x reference)
"""Trainium2 Bass kernel for 3x3 same-padded conv (NCHW) scaled by 1/9.

v6: 1D Winograd F(4,3) along H (vertical), direct conv along W, bf16,
host-side input transform.
  - F(4,3) with Toom-Cook points {0, 2, -2, 1/2, -1/2} (rel err ~7e-3,
    gate 2e-2).  6 products per 4 outputs -> 2x less PE work than direct.
  - Vertical orientation: each PSUM tile M_i is [128, 7 t-tiles, 56 x],
    so the output interleave writes rows 4t+j -> APs with contiguous
    56-element runs instead of stride-4 element interleaves.
  - V = BT d computed ON HOST (fp32) -> bf16, chunk-major layout
    [img, chunk, i, ic_p, ict*tc*58] so every DMA piece is a contiguous
    1.6KB-per-partition run (small pieces otherwise stream at ~half DMA
    rate).  U = G w (1/9 folded); rows i=1..4 scaled by 2 so the output
    transform needs fewer scalar multiplies:
      N1=2M1, N2=2M2, N3=2M3, N4=2M4 in PSUM, and with
      A=N1+N2, B=N1-N2, C=N3+N4, D=N3-N4:
        Y0 = 0.5*(A+C) + N0
        Y1 = B + 0.25*D
        Y2 = 2*(A + 0.0625*C)
        Y3 = 4*(B + 0.015625*D) + N5
  - Engine split per chunk: ScalarE stages M1/M3/M5 out of PSUM + the
    constant scalings; DVE does the PSUM-reading combines + the two
    fused scalar_tensor_tensor folds; GpSimd the SBUF-only adds.
  - DMA: only sync + scalar are HWDGE queues (gpsimd is software-DGE,
    ~2x slower).  Inputs stream on the two HW queues in exact matmul
    consumption order (img0 per-(chunk, i) pieces first); the first four
    output chunks go to the software queue, the rest alternate on the
    HW queues; the last chunk's output is split across both HW queues.
"""

import numpy as np
import ml_dtypes

import concourse.bacc as bacc
import concourse.mybir as mybir
import concourse.tile as tile
from concourse.bass_utils import run_bass_kernel_spmd

N_CORES = 8
N, IC, H, W = 32, 256, 56, 56
OC, KH, KW = 256, 3, 3
NPC = N // N_CORES
ICT = IC // 128
OCT = OC // 128
WP = W + 2
T = H // 4                  # vertical F(4,3) tiles per column (4 rows each)
TC = T // 2                 # tiles per chunk (28 output rows)
NI = 6
MM_I_ORDER = [1, 3, 5, 2, 4, 0]   # i=0 last: only Y0 depends on it

BF16 = mybir.dt.bfloat16
F32 = mybir.dt.float32
MUL = mybir.AluOpType.mult
ADD = mybir.AluOpType.add

BT = np.array([
    [1, 0, -4.25, 0, 1, 0],
    [0, -0.5, -0.25, 2, 1, 0],
    [0, 0.5, -0.25, -2, 1, 0],
    [0, -2, -4, 0.5, 1, 0],
    [0, 2, -4, -0.5, 1, 0],
    [0, 1, 0, -4.25, 0, 1],
], np.float32)
G = np.array([
    [1, 0, 0],
    [1 / 30, 2 / 30, 4 / 30],
    [1 / 30, -2 / 30, 4 / 30],
    [-8 / 15, -4 / 15, -2 / 15],
    [-8 / 15, 4 / 15, -2 / 15],
    [0, 0, 1],
], np.float32)
FOLD = np.array([1, 2, 2, 2, 2, 1], np.float32)

_compiled = None


def _build():
    nc = bacc.Bacc("TRN2", target_bir_lowering=False, debug=False,
                   num_devices=N_CORES)

    # chunk-major, i-major: v[img, ic_p, c, i, ict, tc, x]
    v_d = nc.dram_tensor("v", [NPC, 128, 2, NI, ICT, TC, WP], BF16,
                         kind="ExternalInput")
    u_d = nc.dram_tensor("u", [128, OCT, NI, KW, ICT, 128], BF16,
                         kind="ExternalInput")
    o_d = nc.dram_tensor("out", [NPC, OC, H, W], F32, kind="ExternalOutput")

    with tile.TileContext(nc) as tc:
        with (
            tc.tile_pool(name="vp", bufs=1) as vpool,
            tc.tile_pool(name="up", bufs=1) as upool,
            tc.tile_pool(name="tp", bufs=2) as tpool,
            tc.tile_pool(name="op", bufs=5) as opool,
            tc.tile_pool(name="ps", bufs=8, space="PSUM") as pspool,
        ):
            usb = upool.tile([128, OCT, NI, KW, ICT, 128], BF16, name="usb")
            vt = []
            for img in range(NPC):
                vt.append(vpool.tile([128, 2, NI, ICT, TC, WP], BF16,
                                     tag=f"v{img}", name=f"v{img}"))

            # Few, large input DMAs: Tile recycles ~8 HW-DMA semaphore
            # slots, and each issue blocks until an older DMA fully
            # completes — many small pieces serialize.  The u/v i-axes
            # are stored pre-permuted into matmul order, so the first
            # chunk's data streams as three contiguous thirds per queue.
            # scalar queue: weights in thirds, oct1, img2.
            for k in range(3):
                nc.scalar.dma_start(usb[:, 0, 2 * k:2 * k + 2],
                                    u_d[:, 0, 2 * k:2 * k + 2])
            nc.scalar.dma_start(usb[:, 1], u_d[:, 1])
            nc.scalar.dma_start(vt[2][:], v_d[2])
            # sync queue: img0 chunk A in thirds, chunk B, img1, img3.
            for k in range(3):
                nc.sync.dma_start(vt[0][:, 0, 2 * k:2 * k + 2],
                                  v_d[0, :, 0, 2 * k:2 * k + 2])
            nc.sync.dma_start(vt[0][:, 1], v_d[0, :, 1])
            nc.sync.dma_start(vt[1][:], v_d[1])
            nc.sync.dma_start(vt[3][:], v_d[3])

            # HAM warmup while the first input pieces stream in.
            zs = upool.tile([128, 512], BF16, name="zs")
            nc.vector.memset(zs[:], 0.0)
            zp = pspool.tile([128, 512], F32, tag="pt", name="zp")
            for _ in range(12):
                nc.tensor.matmul(zp[:], zs[:, :128], zs[:], start=True,
                                 stop=True)

            def emit_chunk(img, oct_, c, tlo, thi, ci, last):
                tc = thi - tlo
                # u/v store the i-axis pre-permuted into MM_I_ORDER; pos
                # is the storage index, i the logical transform point.
                # For the final sub-chunk, feed the transform's long
                # dependency chains (via N3/N1) first so only Y0 trails
                # the last matmul group.
                pos_order = [1, 0, 4, 3, 2, 5] if last else list(range(NI))
                pts = {}
                for pos in pos_order:
                    i = MM_I_ORDER[pos]
                    pt = pspool.tile([128, tc, W], F32, tag="pt",
                                     name=f"pt{ci}_{i}")
                    pts[i] = pt
                    first = True
                    for kx in range(KW):
                        for ict in range(ICT):
                            nc.tensor.matmul(
                                pt[:],
                                usb[:, oct_, pos, kx, ict],
                                vt[img][:, c, pos, ict, tlo:thi, kx:kx + W],
                                start=first,
                                stop=(kx == KW - 1 and ict == ICT - 1),
                            )
                            first = False

                ot = opool.tile([128, 4 * tc, W], F32, tag="ot",
                                name=f"ot{ci}")

                def tp(nm):
                    return tpool.tile([128, tc, W], F32, tag=nm,
                                      name=f"{nm}_{ci}")
                s1, s3, c5 = tp("s1"), tp("s3"), tp("c5")
                a, b, cc, dd = tp("a"), tp("b"), tp("cc"), tp("dd")
                g, d4 = tp("g"), tp("d4")
                t2, t3, y3 = tp("t2"), tp("t3"), tp("y3")

                # ScalarE: PSUM staging + constant scalings.  Issue in
                # the order the PSUM tiles land for this chunk.
                if last:
                    nc.scalar.copy(s3[:], pts[3][:])
                    nc.scalar.copy(s1[:], pts[1][:])
                else:
                    nc.scalar.copy(s1[:], pts[1][:])
                    nc.scalar.copy(s3[:], pts[3][:])
                nc.scalar.copy(c5[:], pts[5][:])
                # DVE: PSUM-reading combines + fused folds.
                if last:
                    nc.vector.tensor_add(cc[:], s3[:], pts[4][:])
                    nc.vector.tensor_sub(dd[:], s3[:], pts[4][:])
                    nc.vector.tensor_add(a[:], s1[:], pts[2][:])
                    nc.vector.tensor_sub(b[:], s1[:], pts[2][:])
                else:
                    nc.vector.tensor_add(a[:], s1[:], pts[2][:])
                    nc.vector.tensor_sub(b[:], s1[:], pts[2][:])
                    nc.vector.tensor_add(cc[:], s3[:], pts[4][:])
                    nc.vector.tensor_sub(dd[:], s3[:], pts[4][:])
                if not last:
                    nc.scalar.mul(d4[:], dd[:], 0.25)
                # t2 = A + C/16 ; t3 = B + D/64
                nc.vector.scalar_tensor_tensor(t2[:], cc[:], 0.0625,
                                               a[:], MUL, ADD)
                nc.vector.scalar_tensor_tensor(t3[:], dd[:], 0.015625,
                                               b[:], MUL, ADD)
                # Y2 = 2*t2
                nc.scalar.mul(ot[:, 2::4, :], t2[:], 2.0)
                if last:
                    # Keep the final chunk's chain off GpSimd (its ops
                    # are ~2x slower and would trail the last matmul).
                    nc.gpsimd.tensor_add(g[:], a[:], cc[:])
                    nc.vector.scalar_tensor_tensor(ot[:, 1::4, :], dd[:],
                                                   0.25, b[:], MUL, ADD)
                    nc.vector.scalar_tensor_tensor(ot[:, 0::4, :], g[:],
                                                   0.5, pts[0][:],
                                                   MUL, ADD)
                    nc.vector.scalar_tensor_tensor(ot[:, 3::4, :], t3[:],
                                                   4.0, c5[:], MUL, ADD)
                else:
                    nc.scalar.mul(y3[:], t3[:], 4.0)
                    # GpSimd: SBUF-only adds.
                    nc.gpsimd.tensor_add(g[:], a[:], cc[:])
                    nc.gpsimd.tensor_add(ot[:, 1::4, :], b[:], d4[:])
                    # Y0 = 0.5*g + N0 (gated on last matmul group i=0)
                    nc.vector.scalar_tensor_tensor(ot[:, 0::4, :], g[:],
                                                   0.5, pts[0][:],
                                                   MUL, ADD)
                    nc.gpsimd.tensor_add(ot[:, 3::4, :], y3[:], c5[:])

                ocs = slice(oct_ * 128, (oct_ + 1) * 128)
                y0 = c * 4 * TC + 4 * tlo
                y1 = c * 4 * TC + 4 * thi
                if ci < 6 or ci == 13:
                    # software-DGE queue: separate semaphore pool, keeps
                    # the HW queues (and their 8 sem slots) for inputs
                    # early on and for the final low-latency chunks.
                    nc.gpsimd.dma_start(o_d[img, ocs, y0:y1, :], ot[:])
                elif ci >= 14:
                    # halve the final chunks across both HW queues so
                    # the kernel-ending transfers are short and no issue
                    # op blocks on a recycled DMA semaphore.
                    h = 2 * tc
                    nc.sync.dma_start(o_d[img, ocs, y0:y0 + h, :],
                                      ot[:, :h])
                    nc.scalar.dma_start(o_d[img, ocs, y0 + h:y1, :],
                                        ot[:, h:])
                else:
                    out_eng = nc.sync if ci % 2 == 0 else nc.scalar
                    out_eng.dma_start(o_d[img, ocs, y0:y1, :], ot[:])

            ci = 0
            for img in range(NPC):
                for oct_ in range(OCT):
                    if img == NPC - 1 and oct_ == OCT - 1:
                        # taper the final chunks so the post-matmul
                        # transform tail is short
                        parts = [(0, 0, TC), (1, 0, 4), (1, 4, TC)]
                    else:
                        parts = [(0, 0, TC), (1, 0, TC)]
                    for (c, tlo, thi) in parts:
                        emit_chunk(img, oct_, c, tlo, thi, ci,
                                   last=(c == 1 and thi == TC))
                        ci += 1

    nc.compile()
    return nc


def _get_compiled():
    global _compiled
    if _compiled is None:
        _compiled = _build()
    return _compiled


def _prep_inputs(x, w):
    bf = ml_dtypes.bfloat16
    x = np.asarray(x, dtype=np.float32)
    w = np.asarray(w, dtype=np.float32)

    weff = w / (KH * KW)
    # U[i, kx][oc, ic] = sum_ky G[i, ky] * weff[oc, ic, ky, kx], FOLD-scaled
    U = np.einsum('gy,ocyx->gxoc', G, weff) * FOLD[:, None, None, None]
    U = U[MM_I_ORDER].astype(bf)          # i-axis in matmul order
    u = np.ascontiguousarray(
        U.reshape(NI, KW, OCT, 128, ICT, 128).transpose(5, 2, 0, 1, 4, 3))

    xp = np.zeros((N, IC, H + 2, W + 2), np.float32)
    xp[:, :, 1:H + 1, 1:W + 1] = x
    djs = np.stack([xp[:, :, j:j + 4 * (T - 1) + 1:4, :] for j in range(6)],
                   axis=2)                               # [n, ic, 6, T, WP]
    V = np.einsum('ij,ncjtx->ncitx', BT, djs)
    V = V[:, :, MM_I_ORDER].astype(bf)    # i-axis in matmul order
    # [n, ic, i, t, x] -> [n, ic_p, c, i, ict, tc, x]
    v = np.ascontiguousarray(
        V.reshape(N, ICT, 128, NI, 2, TC, WP).transpose(0, 2, 4, 3, 1, 5, 6))

    return [
        {"v": v[c * NPC:(c + 1) * NPC], "u": u}
        for c in range(N_CORES)
    ]


def kernel(x, w, _trace=False, _trace_kwargs=None):
    nc = _get_compiled()
    in_maps = _prep_inputs(x, w)
    res = run_bass_kernel_spmd(nc, in_maps, list(range(N_CORES)),
                               trace=_trace, **(_trace_kwargs or {}))
    out = np.concatenate([res.results[c]["out"] for c in range(N_CORES)],
                         axis=0)
    if _trace:
        return out, res
    return out


# revision 21
# speedup vs baseline: 1.1506x; 1.1506x over previous
"""Trainium2 Bass kernel for 3x3 same-padded conv (NCHW) scaled by 1/9.

v6: 1D Winograd F(4,3) along H (vertical), direct conv along W, bf16,
host-side input transform.
  - F(4,3) with Toom-Cook points {0, 2, -2, 1/2, -1/2} (rel err ~7e-3,
    gate 2e-2).  6 products per 4 outputs -> 2x less PE work than direct.
  - Vertical orientation: each PSUM tile M_i is [128, 7 t-tiles, 56 x],
    so the output interleave writes rows 4t+j -> APs with contiguous
    56-element runs instead of stride-4 element interleaves.
  - V = BT d computed ON HOST (fp32) -> bf16, chunk-major layout
    [img, chunk, i, ic_p, ict*tc*58] so every DMA piece is a contiguous
    1.6KB-per-partition run (small pieces otherwise stream at ~half DMA
    rate).  U = G w (1/9 folded); rows i=1..4 scaled by 2 so the output
    transform needs fewer scalar multiplies:
      N1=2M1, N2=2M2, N3=2M3, N4=2M4 in PSUM, and with
      A=N1+N2, B=N1-N2, C=N3+N4, D=N3-N4:
        Y0 = 0.5*(A+C) + N0
        Y1 = B + 0.25*D
        Y2 = 2*(A + 0.0625*C)
        Y3 = 4*(B + 0.015625*D) + N5
  - Engine split per chunk: ScalarE stages M1/M3/M5 out of PSUM + the
    constant scalings; DVE does the PSUM-reading combines + the two
    fused scalar_tensor_tensor folds; GpSimd the SBUF-only adds.
  - DMA: only sync + scalar are HWDGE queues (gpsimd is software-DGE,
    ~2x slower).  Inputs stream on the two HW queues in exact matmul
    consumption order (img0 per-(chunk, i) pieces first); the first four
    output chunks go to the software queue, the rest alternate on the
    HW queues; the last chunk's output is split across both HW queues.
"""

import numpy as np
import ml_dtypes

import concourse.bacc as bacc
import concourse.mybir as mybir
import concourse.tile as tile
from concourse.bass_utils import run_bass_kernel_spmd

N_CORES = 8
N, IC, H, W = 32, 256, 56, 56
OC, KH, KW = 256, 3, 3
NPC = N // N_CORES
ICT = IC // 128
OCT = OC // 128
WP = W + 2
T = H // 4                  # vertical F(4,3) tiles per column (4 rows each)
TC = T // 2                 # tiles per chunk (28 output rows)
NI = 6
MM_I_ORDER = [1, 3, 5, 2, 4, 0]   # i=0 last: only Y0 depends on it

BF16 = mybir.dt.bfloat16
F32 = mybir.dt.float32
MUL = mybir.AluOpType.mult
ADD = mybir.AluOpType.add

BT = np.array([
    [1, 0, -4.25, 0, 1, 0],
    [0, -0.5, -0.25, 2, 1, 0],
    [0, 0.5, -0.25, -2, 1, 0],
    [0, -2, -4, 0.5, 1, 0],
    [0, 2, -4, -0.5, 1, 0],
    [0, 1, 0, -4.25, 0, 1],
], np.float32)
G = np.array([
    [1, 0, 0],
    [1 / 30, 2 / 30, 4 / 30],
    [1 / 30, -2 / 30, 4 / 30],
    [-8 / 15, -4 / 15, -2 / 15],
    [-8 / 15, 4 / 15, -2 / 15],
    [0, 0, 1],
], np.float32)
FOLD = np.array([1, 2, 2, 2, 2, 1], np.float32)

_compiled = None


def _build():
    nc = bacc.Bacc("TRN2", target_bir_lowering=False, debug=False,
                   num_devices=N_CORES)

    # chunk-major, i-major: v[img, ic_p, c, i, ict, tc, x]
    v_d = nc.dram_tensor("v", [NPC, 128, 2, NI, ICT, TC, WP], BF16,
                         kind="ExternalInput")
    u_d = nc.dram_tensor("u", [128, OCT, NI, KW, ICT, 128], BF16,
                         kind="ExternalInput")
    o_d = nc.dram_tensor("out", [NPC, OC, H, W], F32, kind="ExternalOutput")

    with tile.TileContext(nc) as tc:
        with (
            tc.tile_pool(name="vp", bufs=1) as vpool,
            tc.tile_pool(name="up", bufs=1) as upool,
            tc.tile_pool(name="tp", bufs=2) as tpool,
            tc.tile_pool(name="op", bufs=5) as opool,
            tc.tile_pool(name="ps", bufs=8, space="PSUM") as pspool,
        ):
            usb = upool.tile([128, OCT, NI, KW, ICT, 128], BF16, name="usb")
            vt = []
            for img in range(NPC):
                vt.append(vpool.tile([128, 2, NI, ICT, TC, WP], BF16,
                                     tag=f"v{img}", name=f"v{img}"))

            # Few, large input DMAs: Tile recycles ~8 HW-DMA semaphore
            # slots, and each issue blocks until an older DMA fully
            # completes — many small pieces serialize.  The u/v i-axes
            # are stored pre-permuted into matmul order, so the first
            # chunk's data streams as three contiguous thirds per queue.
            # scalar queue: weights in halves, oct1, img2.
            for k in range(2):
                nc.scalar.dma_start(usb[:, 0, 3 * k:3 * k + 3],
                                    u_d[:, 0, 3 * k:3 * k + 3])
            nc.scalar.dma_start(usb[:, 1], u_d[:, 1])
            nc.scalar.dma_start(vt[2][:], v_d[2])
            # sync queue: img0 chunk A in halves, chunk B, img1, img3.
            for k in range(2):
                nc.sync.dma_start(vt[0][:, 0, 3 * k:3 * k + 3],
                                  v_d[0, :, 0, 3 * k:3 * k + 3])
            nc.sync.dma_start(vt[0][:, 1], v_d[0, :, 1])
            nc.sync.dma_start(vt[1][:], v_d[1])
            nc.sync.dma_start(vt[3][:], v_d[3])

            # HAM warmup while the first input pieces stream in.
            zs = upool.tile([128, 512], BF16, name="zs")
            nc.vector.memset(zs[:], 0.0)
            zp = pspool.tile([128, 512], F32, tag="pt", name="zp")
            for _ in range(12):
                nc.tensor.matmul(zp[:], zs[:, :128], zs[:], start=True,
                                 stop=True)

            def emit_chunk(img, oct_, c, tlo, thi, ci, last):
                tc = thi - tlo
                # u/v store the i-axis pre-permuted into MM_I_ORDER; pos
                # is the storage index, i the logical transform point.
                # For the final sub-chunk, feed the transform's long
                # dependency chains (via N3/N1) first so only Y0 trails
                # the last matmul group.
                pos_order = [1, 0, 4, 3, 2, 5] if last else list(range(NI))
                pts = {}
                for pos in pos_order:
                    i = MM_I_ORDER[pos]
                    pt = pspool.tile([128, tc, W], F32, tag="pt",
                                     name=f"pt{ci}_{i}")
                    pts[i] = pt
                    first = True
                    for kx in range(KW):
                        for ict in range(ICT):
                            nc.tensor.matmul(
                                pt[:],
                                usb[:, oct_, pos, kx, ict],
                                vt[img][:, c, pos, ict, tlo:thi, kx:kx + W],
                                start=first,
                                stop=(kx == KW - 1 and ict == ICT - 1),
                            )
                            first = False

                ot = opool.tile([128, 4 * tc, W], F32, tag="ot",
                                name=f"ot{ci}")

                def tp(nm):
                    return tpool.tile([128, tc, W], F32, tag=nm,
                                      name=f"{nm}_{ci}")
                s1, s3, c5 = tp("s1"), tp("s3"), tp("c5")
                a, b, cc, dd = tp("a"), tp("b"), tp("cc"), tp("dd")
                g, d4 = tp("g"), tp("d4")
                t2, t3, y3 = tp("t2"), tp("t3"), tp("y3")

                # ScalarE: PSUM staging + constant scalings.  Issue in
                # the order the PSUM tiles land for this chunk.
                if last:
                    nc.scalar.copy(s3[:], pts[3][:])
                    nc.scalar.copy(s1[:], pts[1][:])
                else:
                    nc.scalar.copy(s1[:], pts[1][:])
                    nc.scalar.copy(s3[:], pts[3][:])
                nc.scalar.copy(c5[:], pts[5][:])
                # DVE: PSUM-reading combines + fused folds.
                if last:
                    nc.vector.tensor_add(cc[:], s3[:], pts[4][:])
                    nc.vector.tensor_sub(dd[:], s3[:], pts[4][:])
                    nc.vector.tensor_add(a[:], s1[:], pts[2][:])
                    nc.vector.tensor_sub(b[:], s1[:], pts[2][:])
                else:
                    nc.vector.tensor_add(a[:], s1[:], pts[2][:])
                    nc.vector.tensor_sub(b[:], s1[:], pts[2][:])
                    nc.vector.tensor_add(cc[:], s3[:], pts[4][:])
                    nc.vector.tensor_sub(dd[:], s3[:], pts[4][:])
                if not last:
                    nc.scalar.mul(d4[:], dd[:], 0.25)
                # t2 = A + C/16 ; t3 = B + D/64
                nc.vector.scalar_tensor_tensor(t2[:], cc[:], 0.0625,
                                               a[:], MUL, ADD)
                nc.vector.scalar_tensor_tensor(t3[:], dd[:], 0.015625,
                                               b[:], MUL, ADD)
                # Y2 = 2*t2
                nc.scalar.mul(ot[:, 2::4, :], t2[:], 2.0)
                if last:
                    # Keep the final chunk's chain off GpSimd (its ops
                    # are ~2x slower and would trail the last matmul).
                    nc.gpsimd.tensor_add(g[:], a[:], cc[:])
                    nc.vector.scalar_tensor_tensor(ot[:, 1::4, :], dd[:],
                                                   0.25, b[:], MUL, ADD)
                    nc.vector.scalar_tensor_tensor(ot[:, 0::4, :], g[:],
                                                   0.5, pts[0][:],
                                                   MUL, ADD)
                    nc.vector.scalar_tensor_tensor(ot[:, 3::4, :], t3[:],
                                                   4.0, c5[:], MUL, ADD)
                else:
                    nc.scalar.mul(y3[:], t3[:], 4.0)
                    # GpSimd: SBUF-only adds.
                    nc.gpsimd.tensor_add(g[:], a[:], cc[:])
                    nc.gpsimd.tensor_add(ot[:, 1::4, :], b[:], d4[:])
                    # Y0 = 0.5*g + N0 (gated on last matmul group i=0)
                    nc.vector.scalar_tensor_tensor(ot[:, 0::4, :], g[:],
                                                   0.5, pts[0][:],
                                                   MUL, ADD)
                    nc.gpsimd.tensor_add(ot[:, 3::4, :], y3[:], c5[:])

                ocs = slice(oct_ * 128, (oct_ + 1) * 128)
                y0 = c * 4 * TC + 4 * tlo
                y1 = c * 4 * TC + 4 * thi
                if ci < 6:
                    # software-DGE queue: separate semaphore pool, keeps
                    # the HW queues (and their 8 sem slots) for inputs
                    # early on.
                    nc.gpsimd.dma_start(o_d[img, ocs, y0:y1, :], ot[:])
                elif ci >= 14:
                    # halve the final chunks across both HW queues so
                    # the kernel-ending transfers are short and no issue
                    # op blocks on a recycled DMA semaphore.
                    h = 2 * tc
                    nc.sync.dma_start(o_d[img, ocs, y0:y0 + h, :],
                                      ot[:, :h])
                    nc.scalar.dma_start(o_d[img, ocs, y0 + h:y1, :],
                                        ot[:, h:])
                else:
                    out_eng = nc.sync if ci % 2 == 0 else nc.scalar
                    out_eng.dma_start(o_d[img, ocs, y0:y1, :], ot[:])

            ci = 0
            for img in range(NPC):
                for oct_ in range(OCT):
                    if img == NPC - 1 and oct_ == OCT - 1:
                        # taper the final chunks so the post-matmul
                        # transform tail is short
                        parts = [(0, 0, TC), (1, 0, 4), (1, 4, TC)]
                    else:
                        parts = [(0, 0, TC), (1, 0, TC)]
                    for (c, tlo, thi) in parts:
                        emit_chunk(img, oct_, c, tlo, thi, ci,
                                   last=(c == 1 and thi == TC))
                        ci += 1

    nc.compile()
    return nc


def _get_compiled():
    global _compiled
    if _compiled is None:
        _compiled = _build()
    return _compiled


def _prep_inputs(x, w):
    bf = ml_dtypes.bfloat16
    x = np.asarray(x, dtype=np.float32)
    w = np.asarray(w, dtype=np.float32)

    weff = w / (KH * KW)
    # U[i, kx][oc, ic] = sum_ky G[i, ky] * weff[oc, ic, ky, kx], FOLD-scaled
    U = np.einsum('gy,ocyx->gxoc', G, weff) * FOLD[:, None, None, None]
    U = U[MM_I_ORDER].astype(bf)          # i-axis in matmul order
    u = np.ascontiguousarray(
        U.reshape(NI, KW, OCT, 128, ICT, 128).transpose(5, 2, 0, 1, 4, 3))

    xp = np.zeros((N, IC, H + 2, W + 2), np.float32)
    xp[:, :, 1:H + 1, 1:W + 1] = x
    djs = np.stack([xp[:, :, j:j + 4 * (T - 1) + 1:4, :] for j in range(6)],
                   axis=2)                               # [n, ic, 6, T, WP]
    V = np.einsum('ij,ncjtx->ncitx', BT, djs)
    V = V[:, :, MM_I_ORDER].astype(bf)    # i-axis in matmul order
    # [n, ic, i, t, x] -> [n, ic_p, c, i, ict, tc, x]
    v = np.ascontiguousarray(
        V.reshape(N, ICT, 128, NI, 2, TC, WP).transpose(0, 2, 4, 3, 1, 5, 6))

    return [
        {"v": v[c * NPC:(c + 1) * NPC], "u": u}
        for c in range(N_CORES)
    ]


def kernel(x, w, _trace=False, _trace_kwargs=None):
    nc = _get_compiled()
    in_maps = _prep_inputs(x, w)
    res = run_bass_kernel_spmd(nc, in_maps, list(range(N_CORES)),
                               trace=_trace, **(_trace_kwargs or {}))
    out = np.concatenate([res.results[c]["out"] for c in range(N_CORES)],
                         axis=0)
    if _trace:
        return out, res
    return out
